# revision 32
# baseline (speedup 1.0000x reference)
"""Multi-head causal attention (B=8, T=2048, C=384, H=6, Dh=64) on 8 TRN2 cores.

Sharding: data-parallel over batch - core b computes batch element b end to end
(no collectives).

Pipeline design (vs v1 baseline at ~300us):
- S-score psum tiles hold PAIRS of s-chunks [128, 2, 512] spanning 2 psum
  banks; ONE exp (ACT) instruction covers both chunks -> halves ACT
  per-instruction overhead (240 -> 120 exp instrs).
- exp APs are fringe-trimmed ([.., d0:512]) and PV matmuls stream only the
  causal columns (N = 512-d), removing the P memsets entirely.
- software-pipelined issue order per head: S(p) / exp(p) / PV(p-1) so the
  tensor engine always has queued work while ACT computes exp.
- FILLER QUEUE: all non-attention tensor work (QKV projection chains and
  fused output-projection steps) is chopped into small closures drained
  one-per-pair inside the attention inner loop.  The attention loop alone
  is ACT-paced (~1.04us/pair vs ~0.85us of PE work per pair), which idles
  the PE and drops its p-state clock from 2.4 to 1.2 GHz; the fillers keep
  the PE dense so everything runs at full clock.
- NORMALIZED attT: per head, the softmax denominator row (augmented-V row
  64 of O) is reciprocal'd straight out of PSUM (DVE), broadcast across 64
  partitions (GPSIMD partition_broadcast), and multiplied into the O->attT
  staging copy.  attT is then already normalized, so the output projection
  fuses all 6 heads into K=128 matmul chains: 3 accumulating matmuls + one
  bias-add per 128-row tile (48 matmuls total vs 96 K=64 matmuls + 96
  serial scalar_tensor_tensor ops), and no denominator transposes exist.

Per-core layout (all "T" means transposed, head-dim/channel on partitions):
  xT   [128, 3, 2048]  bf16   c = 128*ci + p
  wq/wk[128, 3, 384]   bf16   packed Wq[h,c,d] -> [c, h*64+d]
  wv   [128, 3, 384]   bf16
  wp   [128, 3, 384]   bf16   Wp[c, e] -> [128, ci, e]
  biasb[128, 384]      f32    bias broadcast rows
  QT/KT/attT [128, 3, 2048] bf16  (hd = 128*bi + po + d, po = (h%2)*64)
  Vt   [128, 16, 6, 65] bf16  V augmented with ones col (softmax denom row)
"""

import numpy as np
import ml_dtypes

import concourse.bass as bass
import concourse.tile as tile
from concourse import bacc, mybir
from concourse.bass import ts, ds

F32 = mybir.dt.float32
BF16 = mybir.dt.bfloat16
AF = mybir.ActivationFunctionType
ALU = mybir.AluOpType

B, T, C = 8, 2048, 384
H, DH = 6, 64
SCALE = DH ** -0.5
NCORES = 8
TJ = 512            # q-block width
NJ = T // TJ        # 4 q-blocks
SC = 128            # s-chunk
NQ = TJ // SC       # q-sub-chunks / s-chunks per block (4)
NCI = C // 128      # 3 channel chunks


def build_kernel():
    nc = bacc.Bacc("TRN2", target_bir_lowering=False, debug=False)

    xT_d = nc.dram_tensor("xT", [128, NCI, T], BF16, kind="ExternalInput").ap()
    wq_d = nc.dram_tensor("wq", [128, NCI, C], BF16, kind="ExternalInput").ap()
    wk_d = nc.dram_tensor("wk", [128, NCI, C], BF16, kind="ExternalInput").ap()
    wv_d = nc.dram_tensor("wv", [128, NCI, C], BF16, kind="ExternalInput").ap()
    wp_d = nc.dram_tensor("wp", [128, NCI, C], BF16, kind="ExternalInput").ap()
    biasb_d = nc.dram_tensor("biasb", [128, C], F32, kind="ExternalInput").ap()
    y_d = nc.dram_tensor("y", [T, C], F32, kind="ExternalOutput").ap()

    with tile.TileContext(nc) as tc:
        with tc.tile_pool(name="const", bufs=1) as cpool, \
             tc.tile_pool(name="ps", bufs=1, space="PSUM") as ps, \
             tc.tile_pool(name="pp", bufs=3) as ppool, \
             tc.tile_pool(name="yp", bufs=2) as ypool:
            xT = cpool.tile([128, NCI, T], BF16)
            wq = cpool.tile([128, NCI, C], BF16)
            wk = cpool.tile([128, NCI, C], BF16)
            wv = cpool.tile([128, NCI, C], BF16)
            wp = cpool.tile([128, NCI, C], BF16)
            biasb = cpool.tile([128, C], F32)
            QT = cpool.tile([128, NCI, T], BF16)
            KT = cpool.tile([128, NCI, T], BF16)
            attT = cpool.tile([128, NCI, T], BF16)
            Vt = cpool.tile([128, 16, H, 65], BF16)
            onesb = cpool.tile([128, 16 * H], BF16)

            # block-0's working set first so compute starts ~4us earlier
            nc.sync.dma_start(wq[:], wq_d[:])
            nc.sync.dma_start(wk[:], wk_d[:])
            for ci in range(NCI):
                nc.sync.dma_start(xT[:, ci, 0:TJ], xT_d[:, ci, 0:TJ])
            for ci in range(NCI):
                nc.sync.dma_start(xT[:, ci, TJ:T], xT_d[:, ci, TJ:T])
            nc.sync.dma_start(wv[:], wv_d[:])
            nc.sync.dma_start(wp[:], wp_d[:])
            nc.sync.dma_start(biasb[:], biasb_d[:])
            # augmented-ones column of Vt (col 64 of each head slot)
            nc.gpsimd.memset(onesb[:], 1.0)
            nc.vector.tensor_copy(
                Vt[:, :, :, 64:65],
                onesb[:].rearrange("p (a b c) -> p a b c", a=16, b=H),
            )

            # deferred small PE work units, drained one per attention pair
            filler_q = []  # (label, closure)

            def drain(n=1):
                for _ in range(n):
                    if not filler_q:
                        return
                    filler_q.pop(0)[1]()

            def drain_auto():
                drain(1)

            def drain_all():
                while filler_q:
                    filler_q.pop(0)[1]()

            def drain_label(lbl):
                # force-drain (in order) until no closures tagged lbl remain
                while any(l == lbl for l, _ in filler_q):
                    drain(1)

            def enqueue_proj(jb):
                """QT/KT for t-block jb; V rows for s-chunks 4jb..4jb+3."""
                def qk_chain(dst, w, pi):
                    def run():
                        pt = ps.tile([128, TJ], F32, tag="mm", bufs=2,
                                     name=f"pqk{jb}{pi}")
                        for ci in range(NCI):
                            nc.tensor.matmul(
                                pt[:],
                                lhsT=w[:, ci, ts(pi, 128)],
                                rhs=xT[:, ci, ts(jb, TJ)],
                                start=(ci == 0), stop=(ci == NCI - 1),
                            )
                        nc.vector.tensor_copy(dst[:, pi, ts(jb, TJ)], pt[:])
                    return run

                def v_chain(si):
                    def run():
                        pt = ps.tile([128, C], F32, tag="mm", bufs=2,
                                     name=f"pv{si}")
                        for ci in range(NCI):
                            nc.tensor.matmul(
                                pt[:],
                                lhsT=xT[:, ci, ts(si, 128)],
                                rhs=wv[:, ci, :],
                                start=(ci == 0), stop=(ci == NCI - 1),
                            )
                        nc.vector.tensor_copy(
                            Vt[:, si, :, 0:64],
                            pt[:].rearrange("p (h d) -> p h d", h=H),
                        )
                    return run

                filler_q.append((("projp", jb, 0), qk_chain(QT, wq, 0)))
                filler_q.append((("projp", jb, 0), qk_chain(KT, wk, 0)))
                for si in range(NQ * jb, NQ * jb + NQ):
                    filler_q.append((("projv", jb), v_chain(si)))
                for pi in (1, 2):
                    filler_q.append((("projp", jb, pi), qk_chain(QT, wq, pi)))
                    filler_q.append((("projp", jb, pi), qk_chain(KT, wk, pi)))

            def attention_head(j, h, drecip, pre_pv=None):
                po = (h % 2) * 64
                bi = h // 2
                nch = NQ * j + NQ       # s-chunks (always even)
                npair = nch // 2
                O = ps.tile([65, TJ], F32, tag="O", bufs=2, name=f"O{j}{h}")
                sps_t = [None] * npair
                P_t = [None] * npair

                def off(i):
                    return SC * i - TJ * j if i >= NQ * j else 0

                def S_pair(p):
                    spt = ps.tile([128, 2, TJ], F32, tag="sp", bufs=2,
                                  name=f"sp{j}{h}{p}")
                    sps_t[p] = spt
                    for c in (0, 1):
                        i = 2 * p + c
                        d = off(i)
                        nc.tensor.matmul(
                            spt[:, c, d:TJ],
                            lhsT=KT[po:po + 64, bi, ts(i, SC)],
                            rhs=QT[po:po + 64, bi, ds(j * TJ + d, TJ - d)],
                            start=True, stop=True,
                        )

                def EXP_pair(p):
                    d0 = off(2 * p)
                    pt = ppool.tile([128, 2, TJ], BF16, tag="P",
                                    name=f"P{j}{h}{p}")
                    P_t[p] = pt
                    nc.scalar.activation(pt[:, :, d0:TJ], sps_t[p][:, :, d0:TJ],
                                         AF.Exp, scale=SCALE)
                    for c in (0, 1):
                        i = 2 * p + c
                        if i >= NQ * j:  # diagonal window mask
                            d = off(i)
                            nc.gpsimd.affine_select(
                                out=pt[:, c, d:d + 128], in_=pt[:, c, d:d + 128],
                                pattern=[[1, 128]],
                                compare_op=ALU.is_ge,
                                fill=0.0, base=0, channel_multiplier=-1,
                            )

                def PV_pair(p):
                    for c in (0, 1):
                        i = 2 * p + c
                        d = off(i)
                        nc.tensor.matmul(
                            O[:, d:TJ],
                            lhsT=Vt[:, i, h, :],
                            rhs=P_t[p][:, c, d:TJ],
                            start=(i == 0), stop=(i == nch - 1),
                        )

                # S/exp run TWO pairs ahead of PV: ACT always has queued exp
                # work, so filler closures never starve it, and PV never
                # waits on an exp that hasn't started
                S_pair(0)
                EXP_pair(0)
                if npair > 1:
                    S_pair(1)
                    EXP_pair(1)
                if pre_pv is not None:
                    pre_pv()
                for p in range(2, npair):
                    drain_auto()
                    PV_pair(p - 2)
                    S_pair(p)
                    EXP_pair(p)
                if npair > 1:
                    drain_auto()
                    PV_pair(npair - 2)
                drain_auto()
                PV_pair(npair - 1)
                # normalize while staging: attT = O[0:64] * (1/denom_row)
                # (approx recip: ~18 bits, far beyond attT's bf16 mantissa;
                # denominators are in [1, ~5e3] so no edge cases.  The
                # custom-DVE op needs an SBUF source, so stage the row.)
                drow = ypool.tile([1, TJ], F32, tag="drow", name=f"dr{j}{h}")
                nc.vector.tensor_copy(drow[0:1, :], O[64:65, :])
                nc.vector.reciprocal_approx_fast(drecip[0:1, h, :],
                                                 drow[0:1, :])
                rb = ypool.tile([64, TJ], F32, tag="rb", name=f"rb{j}{h}")
                nc.gpsimd.partition_broadcast(rb[0:64, :], drecip[0:1, h, :])
                nc.vector.tensor_tensor(
                    out=attT[po:po + 64, bi, ts(j, TJ)],
                    in0=O[0:64, :], in1=rb[0:64, :], op=ALU.mult)

            def enqueue_out_proj(j):
                """Fused output projection for block j: per 128-row tile,
                3 K=128 accumulating matmuls over the normalized attT, then
                one bias-add to SBUF and the output DMA."""
                def u_step(q):
                    def run():
                        tb = NQ * j + q
                        Yp = ps.tile([128, C], F32, tag="mm", bufs=2,
                                     name=f"Yp{tb}")
                        for bi in range(NCI):
                            nc.tensor.matmul(
                                Yp[:],
                                lhsT=attT[:, bi, ts(tb, 128)],
                                rhs=wp[:, bi, :],
                                start=(bi == 0), stop=(bi == NCI - 1),
                            )
                        Ye = ypool.tile([128, C], F32, tag="Ye", name=f"Ye{tb}")
                        nc.vector.tensor_add(out=Ye[:], in0=Yp[:], in1=biasb[:])
                        nc.sync.dma_start(y_d[ts(tb, 128), :], Ye[:])
                    return run

                for q in range(NQ):
                    filler_q.append((("outp", j), u_step(q)))

            # ---- main schedule ----
            enqueue_proj(0)
            for j in range(NJ):
                drecip = ypool.tile([1, H, TJ], F32, tag="drecip",
                                    name=f"drc{j}")
                if j == 0:
                    # run ALL block-0 projections up-front: ACT has nothing
                    # to do yet, and any filler between block-0 S pairs
                    # creates an ACT lag the PE pays back at block 1
                    drain_label(("projp", 0, 2))
                for h in range(H):
                    # guard: this head's QT/KT channel chunk must be issued;
                    # the block's V rows are only needed by the first PV
                    drain_label(("projp", j, h // 2))
                    if j > 0 and h == 0:
                        enqueue_out_proj(j - 1)
                    pre_pv = None
                    if h == 0:
                        pre_pv = lambda jj=j: drain_label(("projv", jj))
                    attention_head(j, h, drecip, pre_pv)
                    # block 0 runs filler-free between its S pairs (any
                    # filler there delays the exp feed and ACT never
                    # recovers); later blocks spread fillers from h==1
                    if j + 1 < NJ and h == (5 if j == 0 else 1):
                        enqueue_proj(j + 1)
            enqueue_out_proj(NJ - 1)
            drain_all()

    nc.compile()
    return nc


def _prep_inputs(x, Wq, Wk, Wv, Wp, bp):
    """Host-side shard + layout prep. Returns per-core input maps."""
    bf = ml_dtypes.bfloat16
    x = np.asarray(x, dtype=np.float32)

    def pack_w(W):  # [H, C, Dh] -> [128, NCI, H*Dh]
        Whd = np.transpose(np.asarray(W, np.float32), (1, 0, 2)).reshape(C, H * DH)
        return np.ascontiguousarray(
            Whd.reshape(NCI, 128, H * DH).transpose(1, 0, 2)
        ).astype(bf)

    wq_p, wk_p, wv_p = pack_w(Wq), pack_w(Wk), pack_w(Wv)
    wp_p = np.ascontiguousarray(
        np.asarray(Wp, np.float32).reshape(NCI, 128, C).transpose(1, 0, 2)
    ).astype(bf)

    biasb = np.broadcast_to(np.asarray(bp, np.float32), (128, C)).copy()

    in_maps = []
    for b in range(B):
        xT = np.ascontiguousarray(
            x[b].T.reshape(NCI, 128, T).transpose(1, 0, 2)
        ).astype(bf)
        in_maps.append({
            "xT": xT, "wq": wq_p, "wk": wk_p, "wv": wv_p, "wp": wp_p,
            "biasb": biasb,
        })
    return in_maps


_CACHE = {}


def kernel(x, Wq, Wk, Wv, Wp, bp):
    from concourse.bass_utils import run_bass_kernel_spmd

    if "nc" not in _CACHE:
        _CACHE["nc"] = build_kernel()
    nc = _CACHE["nc"]
    in_maps = _prep_inputs(x, Wq, Wk, Wv, Wp, bp)
    res = run_bass_kernel_spmd(nc, in_maps, list(range(NCORES)))
    out = np.stack([res.results[b]["y"] for b in range(B)], axis=0)
    return out.astype(np.float32)


# revision 34
# speedup vs baseline: 1.0544x; 1.0544x over previous
"""Multi-head causal attention (B=8, T=2048, C=384, H=6, Dh=64) on 8 TRN2 cores.

Sharding: data-parallel over batch - core b computes batch element b end to end
(no collectives).

Pipeline design (vs v1 baseline at ~300us):
- S-score psum tiles hold PAIRS of s-chunks [128, 2, 512] spanning 2 psum
  banks; ONE exp (ACT) instruction covers both chunks -> halves ACT
  per-instruction overhead (240 -> 120 exp instrs).
- exp APs are fringe-trimmed ([.., d0:512]) and PV matmuls stream only the
  causal columns (N = 512-d), removing the P memsets entirely.
- software-pipelined issue order per head: S(p) / exp(p) / PV(p-1) so the
  tensor engine always has queued work while ACT computes exp.
- FILLER QUEUE: all non-attention tensor work (QKV projection chains and
  fused output-projection steps) is chopped into small closures drained
  one-per-pair inside the attention inner loop.  The attention loop alone
  is ACT-paced (~1.04us/pair vs ~0.85us of PE work per pair), which idles
  the PE and drops its p-state clock from 2.4 to 1.2 GHz; the fillers keep
  the PE dense so everything runs at full clock.
- NORMALIZED attT: per head, the softmax denominator row (augmented-V row
  64 of O) is reciprocal'd straight out of PSUM (DVE), broadcast across 64
  partitions (GPSIMD partition_broadcast), and multiplied into the O->attT
  staging copy.  attT is then already normalized, so the output projection
  fuses all 6 heads into K=128 matmul chains: 3 accumulating matmuls + one
  bias-add per 128-row tile (48 matmuls total vs 96 K=64 matmuls + 96
  serial scalar_tensor_tensor ops), and no denominator transposes exist.

Per-core layout (all "T" means transposed, head-dim/channel on partitions):
  xT   [128, 3, 2048]  bf16   c = 128*ci + p
  wq/wk[128, 3, 384]   bf16   packed Wq[h,c,d] -> [c, h*64+d]
  wv   [128, 3, 384]   bf16
  wp   [128, 3, 384]   bf16   Wp[c, e] -> [128, ci, e]
  biasb[128, 384]      f32    bias broadcast rows
  QT/KT/attT [128, 3, 2048] bf16  (hd = 128*bi + po + d, po = (h%2)*64)
  Vt   [128, 16, 6, 65] bf16  V augmented with ones col (softmax denom row)
"""

import numpy as np
import ml_dtypes

import concourse.bass as bass
import concourse.tile as tile
from concourse import bacc, mybir
from concourse.bass import ts, ds

F32 = mybir.dt.float32
BF16 = mybir.dt.bfloat16
AF = mybir.ActivationFunctionType
ALU = mybir.AluOpType

B, T, C = 8, 2048, 384
H, DH = 6, 64
SCALE = DH ** -0.5
NCORES = 8
TJ = 512            # q-block width
NJ = T // TJ        # 4 q-blocks
SC = 128            # s-chunk
NQ = TJ // SC       # q-sub-chunks / s-chunks per block (4)
NCI = C // 128      # 3 channel chunks


def build_kernel():
    nc = bacc.Bacc("TRN2", target_bir_lowering=False, debug=False)

    xT_d = nc.dram_tensor("xT", [128, NCI, T], BF16, kind="ExternalInput").ap()
    wq_d = nc.dram_tensor("wq", [128, NCI, C], BF16, kind="ExternalInput").ap()
    wk_d = nc.dram_tensor("wk", [128, NCI, C], BF16, kind="ExternalInput").ap()
    wv_d = nc.dram_tensor("wv", [128, NCI, C], BF16, kind="ExternalInput").ap()
    wp_d = nc.dram_tensor("wp", [128, NCI, C], BF16, kind="ExternalInput").ap()
    biasb_d = nc.dram_tensor("biasb", [128, C], F32, kind="ExternalInput").ap()
    y_d = nc.dram_tensor("y", [T, C], F32, kind="ExternalOutput").ap()

    with tile.TileContext(nc) as tc:
        with tc.tile_pool(name="const", bufs=1) as cpool, \
             tc.tile_pool(name="ps", bufs=1, space="PSUM") as ps, \
             tc.tile_pool(name="pp", bufs=3) as ppool, \
             tc.tile_pool(name="yp", bufs=2) as ypool:
            xT = cpool.tile([128, NCI, T], BF16)
            wq = cpool.tile([128, NCI, C], BF16)
            wk = cpool.tile([128, NCI, C], BF16)
            wv = cpool.tile([128, NCI, C], BF16)
            wp = cpool.tile([128, NCI, C], BF16)
            biasb = cpool.tile([128, C], F32)
            QT = cpool.tile([128, NCI, T], BF16)
            KT = cpool.tile([128, NCI, T], BF16)
            attT = cpool.tile([128, NCI, T], BF16)
            Vt = cpool.tile([128, 16, H, 65], BF16)
            onesb = cpool.tile([128, 16 * H], BF16)

            # block-0's working set first so compute starts ~4us earlier
            nc.sync.dma_start(wq[:], wq_d[:])
            nc.sync.dma_start(wk[:], wk_d[:])
            for ci in range(NCI):
                nc.sync.dma_start(xT[:, ci, 0:TJ], xT_d[:, ci, 0:TJ])
            for ci in range(NCI):
                nc.sync.dma_start(xT[:, ci, TJ:T], xT_d[:, ci, TJ:T])
            nc.sync.dma_start(wv[:], wv_d[:])
            nc.sync.dma_start(wp[:], wp_d[:])
            nc.sync.dma_start(biasb[:], biasb_d[:])
            # augmented-ones column of Vt (col 64 of each head slot)
            nc.gpsimd.memset(onesb[:], 1.0)
            nc.vector.tensor_copy(
                Vt[:, :, :, 64:65],
                onesb[:].rearrange("p (a b c) -> p a b c", a=16, b=H),
            )

            # deferred small PE work units, drained one per attention pair
            filler_q = []  # (label, closure)

            def drain(n=1):
                for _ in range(n):
                    if not filler_q:
                        return
                    filler_q.pop(0)[1]()

            def drain_auto():
                drain(1)

            def drain_all():
                while filler_q:
                    filler_q.pop(0)[1]()

            def drain_label(lbl):
                # force-drain (in order) until no closures tagged lbl remain
                while any(l == lbl for l, _ in filler_q):
                    drain(1)

            def enqueue_proj(jb):
                """QT/KT for t-block jb; V rows for s-chunks 4jb..4jb+3."""
                def qk_chain(dst, w, pi):
                    def run():
                        pt = ps.tile([128, TJ], F32, tag="mm", bufs=2,
                                     name=f"pqk{jb}{pi}")
                        for ci in range(NCI):
                            nc.tensor.matmul(
                                pt[:],
                                lhsT=w[:, ci, ts(pi, 128)],
                                rhs=xT[:, ci, ts(jb, TJ)],
                                start=(ci == 0), stop=(ci == NCI - 1),
                            )
                        nc.vector.tensor_copy(dst[:, pi, ts(jb, TJ)], pt[:])
                    return run

                def v_chain(si):
                    def run():
                        pt = ps.tile([128, C], F32, tag="mm", bufs=2,
                                     name=f"pv{si}")
                        for ci in range(NCI):
                            nc.tensor.matmul(
                                pt[:],
                                lhsT=xT[:, ci, ts(si, 128)],
                                rhs=wv[:, ci, :],
                                start=(ci == 0), stop=(ci == NCI - 1),
                            )
                        nc.vector.tensor_copy(
                            Vt[:, si, :, 0:64],
                            pt[:].rearrange("p (h d) -> p h d", h=H),
                        )
                    return run

                filler_q.append((("projp", jb, 0), qk_chain(QT, wq, 0)))
                filler_q.append((("projp", jb, 0), qk_chain(KT, wk, 0)))
                for si in range(NQ * jb, NQ * jb + NQ):
                    filler_q.append((("projv", jb), v_chain(si)))
                for pi in (1, 2):
                    filler_q.append((("projp", jb, pi), qk_chain(QT, wq, pi)))
                    filler_q.append((("projp", jb, pi), qk_chain(KT, wk, pi)))

            def attention_head(j, h, drecip, pre_pv=None):
                po = (h % 2) * 64
                bi = h // 2
                nch = NQ * j + NQ       # s-chunks (always even)
                npair = nch // 2
                O = ps.tile([65, TJ], F32, tag="O", bufs=2, name=f"O{j}{h}")
                sps_t = [None] * npair
                P_t = [None] * npair

                def off(i):
                    return SC * i - TJ * j if i >= NQ * j else 0

                def S_pair(p):
                    spt = ps.tile([128, 2, TJ], F32, tag="sp", bufs=2,
                                  name=f"sp{j}{h}{p}")
                    sps_t[p] = spt
                    for c in (0, 1):
                        i = 2 * p + c
                        d = off(i)
                        nc.tensor.matmul(
                            spt[:, c, d:TJ],
                            lhsT=KT[po:po + 64, bi, ts(i, SC)],
                            rhs=QT[po:po + 64, bi, ds(j * TJ + d, TJ - d)],
                            start=True, stop=True,
                        )

                def EXP_pair(p):
                    d0 = off(2 * p)
                    pt = ppool.tile([128, 2, TJ], BF16, tag="P",
                                    name=f"P{j}{h}{p}")
                    P_t[p] = pt
                    nc.scalar.activation(pt[:, :, d0:TJ], sps_t[p][:, :, d0:TJ],
                                         AF.Exp, scale=SCALE)
                    for c in (0, 1):
                        i = 2 * p + c
                        if i >= NQ * j:  # diagonal window mask
                            d = off(i)
                            nc.gpsimd.affine_select(
                                out=pt[:, c, d:d + 128], in_=pt[:, c, d:d + 128],
                                pattern=[[1, 128]],
                                compare_op=ALU.is_ge,
                                fill=0.0, base=0, channel_multiplier=-1,
                            )

                def PV_pair(p):
                    for c in (0, 1):
                        i = 2 * p + c
                        d = off(i)
                        nc.tensor.matmul(
                            O[:, d:TJ],
                            lhsT=Vt[:, i, h, :],
                            rhs=P_t[p][:, c, d:TJ],
                            start=(i == 0), stop=(i == nch - 1),
                        )

                # S/exp run TWO pairs ahead of PV: ACT always has queued exp
                # work, so filler closures never starve it, and PV never
                # waits on an exp that hasn't started
                S_pair(0)
                EXP_pair(0)
                if npair > 1:
                    S_pair(1)
                    EXP_pair(1)
                if pre_pv is not None:
                    pre_pv()
                for p in range(2, npair):
                    drain_auto()
                    PV_pair(p - 2)
                    S_pair(p)
                    EXP_pair(p)
                if npair > 1:
                    drain_auto()
                    PV_pair(npair - 2)
                drain_auto()
                PV_pair(npair - 1)
                # normalize while staging: attT = O[0:64] * (1/denom_row)
                # (approx recip: ~18 bits, far beyond attT's bf16 mantissa;
                # denominators are in [1, ~5e3] so no edge cases.  The
                # custom-DVE op needs an SBUF source, so stage the row.)
                drow = ypool.tile([1, TJ], F32, tag="drow", name=f"dr{j}{h}")
                nc.vector.tensor_copy(drow[0:1, :], O[64:65, :])
                nc.vector.reciprocal_approx_fast(drecip[0:1, h, :],
                                                 drow[0:1, :])
                rb = ypool.tile([64, TJ], F32, tag="rb", name=f"rb{j}{h}")
                nc.gpsimd.partition_broadcast(rb[0:64, :], drecip[0:1, h, :])
                nc.vector.tensor_tensor(
                    out=attT[po:po + 64, bi, ts(j, TJ)],
                    in0=O[0:64, :], in1=rb[0:64, :], op=ALU.mult)

            def enqueue_out_proj(j):
                """Fused output projection for block j: per 128-row tile,
                3 K=128 accumulating matmuls over the normalized attT, then
                one bias-add to SBUF and the output DMA."""
                def u_step(q):
                    def run():
                        tb = NQ * j + q
                        Yp = ps.tile([128, C], F32, tag="mm", bufs=2,
                                     name=f"Yp{tb}")
                        for bi in range(NCI):
                            nc.tensor.matmul(
                                Yp[:],
                                lhsT=attT[:, bi, ts(tb, 128)],
                                rhs=wp[:, bi, :],
                                start=(bi == 0), stop=(bi == NCI - 1),
                            )
                        Ye = ypool.tile([128, C], F32, tag="Ye", name=f"Ye{tb}")
                        nc.vector.tensor_add(out=Ye[:], in0=Yp[:], in1=biasb[:])
                        nc.sync.dma_start(y_d[ts(tb, 128), :], Ye[:])
                    return run

                for q in range(NQ):
                    filler_q.append((("outp", j), u_step(q)))

            # ---- main schedule ----
            enqueue_proj(0)
            for j in range(NJ):
                drecip = ypool.tile([1, H, TJ], F32, tag="drecip",
                                    name=f"drc{j}")
                if j == 0:
                    # run ALL block-0 projections up-front: ACT has nothing
                    # to do yet, and any filler between block-0 S pairs
                    # creates an ACT lag the PE pays back at block 1
                    drain_label(("projp", 0, 2))
                for h in range(H):
                    # guard: this head's QT/KT channel chunk must be issued;
                    # the block's V rows are only needed by the first PV
                    drain_label(("projp", j, h // 2))
                    if j > 0 and h == 0:
                        enqueue_out_proj(j - 1)
                    pre_pv = None
                    if h == 0:
                        pre_pv = lambda jj=j: drain_label(("projv", jj))
                    attention_head(j, h, drecip, pre_pv)
                    # block 0 runs filler-free between its S pairs (any
                    # filler there delays the exp feed and ACT never
                    # recovers); later blocks spread fillers from h==1
                    if j + 1 < NJ and h == (5 if j == 0 else 1):
                        enqueue_proj(j + 1)
            enqueue_out_proj(NJ - 1)
            drain_all()

    nc.compile()
    return nc


def _prep_inputs(x, Wq, Wk, Wv, Wp, bp):
    """Host-side shard + layout prep. Returns per-core input maps."""
    bf = ml_dtypes.bfloat16
    x = np.asarray(x, dtype=np.float32)

    def pack_w(W):  # [H, C, Dh] -> [128, NCI, H*Dh]
        Whd = np.transpose(np.asarray(W, np.float32), (1, 0, 2)).reshape(C, H * DH)
        return np.ascontiguousarray(
            Whd.reshape(NCI, 128, H * DH).transpose(1, 0, 2)
        ).astype(bf)

    wq_p, wk_p, wv_p = pack_w(Wq), pack_w(Wk), pack_w(Wv)
    wp_p = np.ascontiguousarray(
        np.asarray(Wp, np.float32).reshape(NCI, 128, C).transpose(1, 0, 2)
    ).astype(bf)

    biasb = np.broadcast_to(np.asarray(bp, np.float32), (128, C)).copy()

    in_maps = []
    for b in range(B):
        xT = np.ascontiguousarray(
            x[b].T.reshape(NCI, 128, T).transpose(1, 0, 2)
        ).astype(bf)
        in_maps.append({
            "xT": xT, "wq": wq_p, "wk": wk_p, "wv": wv_p, "wp": wp_p,
            "biasb": biasb,
        })
    return in_maps


_CACHE = {}


def kernel(x, Wq, Wk, Wv, Wp, bp):
    from concourse.bass_utils import run_bass_kernel_spmd

    if "nc" not in _CACHE:
        _CACHE["nc"] = build_kernel()
    nc = _CACHE["nc"]
    in_maps = _prep_inputs(x, Wq, Wk, Wv, Wp, bp)
    res = run_bass_kernel_spmd(nc, in_maps, list(range(NCORES)))
    out = np.stack([res.results[b]["y"] for b in range(B)], axis=0)
    return out.astype(np.float32)
